# revision 5
# baseline (speedup 1.0000x reference)
"""CTC loss (nn_CTCLoss) on 8 Trainium2 NeuronCores — pure batch data-parallel.

kernel(predicts [256,160,6625] f32 log-probs, labels [256,25] i32,
       label_lengths [256]) -> scalar f32 mean CTC loss.

Sharding: batch 256 -> 8 cores x 32.  Each core runs the full T=160 forward
scan on its shard; host averages the 8x32 per-sample losses.

Per-core pipeline (one SPMD program):
  1. Stream the predicts shard [32,160,6625] f32 through SBUF in 40 tiles
     [128, 6625] laid out (j, b, u): j = t//32, 4 batches per tile, u = t%32
     (partition = 32*(b%4) + u).  HWDGE DMA, 3.4 MB per transfer.
  2. GPSIMD ap_gather pulls the 51 extended-label columns (padded to 64)
     per batch from each tile; 16-partition groups align with batches.
  3. ACT exp with bias: p = exp(log_p + BIAS) into stage[j].
  4. Two DVE 32x32 stream-transpose passes: stage[j] ([u,s] per b) ->
     psT[j] ([s, u*32+b]) -> pbig[j] ([b, u*64+s]).
  5. DVE scan over t in probability space:
       alpha'[s] = (alpha[s] + alpha[s-1] + skip[s]*alpha[s-2]) * p_t[s]
     with per-sample max-renormalization every RENORM steps (log accum).
  6. loss_b = BIAS*T - (ln(sum_{s in {2l, 2l-1}} alpha[s]) + acc).
"""

import numpy as np

import concourse.bass as bass
import concourse.mybir as mybir
import concourse.tile as tile
from concourse import bacc, library_config
from concourse.bass_utils import run_bass_kernel_spmd

F32 = mybir.dt.float32
I16 = mybir.dt.int16

N_CORES = 8
B_FULL = 256
B_LOC = 32      # batch per core
T = 160
C = 6625
S = 25
L = 2 * S + 1   # 51
SP = 64         # padded extended-label dim
NJ = 5          # t-blocks of 32
NBQ = 8         # batch quads per t-block
NTILES = NJ * NBQ
BIAS = 8.8
RENORM = 16


def _prep_core_inputs(pred, labels, lens):
    """One core's shard -> device input dict."""
    ext64 = np.zeros((B_LOC, SP), dtype=np.int64)
    ext64[:, 1:L:2] = labels.astype(np.int64)

    prev2 = np.full((B_LOC, SP), -1, dtype=np.int64)
    prev2[:, 2:] = ext64[:, :-2]
    mskip = ((ext64 != 0) & (ext64 != prev2)).astype(np.float32)
    mskip[:, L:] = 0.0

    minit = np.zeros((B_LOC, SP), dtype=np.float32)
    minit[:, 0:2] = 1.0

    mfin = np.zeros((B_LOC, SP), dtype=np.float32)
    ll = lens.astype(np.int64)
    for b in range(B_LOC):
        mfin[b, 2 * ll[b]] = 1.0
        mfin[b, 2 * ll[b] - 1] = 1.0

    # viability pruning: zero positions that can no longer reach the final
    # states {2len-1, 2len} (max advance 2/step), plus the s >= L pads.
    # Keeps the renorm max tracking contributing paths so the final values
    # never sink into the f32 denormal range (TRN2 flushes denormals).
    # Also prune s > 2len (beyond the final state): such mass can never flow
    # back down, so this is exact — and it keeps the final renorm max equal
    # to the final-position values, so the last Ln sees an O(1) input (the
    # ACT Ln table saturates for inputs below ~1e-20).
    s_idx = np.arange(SP)
    t_idx = np.arange(T)
    smin = (2 * ll[:, None] - 1 - 2 * (T - 1 - t_idx))[:, :, None]  # [B,T,1]
    smax = (2 * ll)[:, None, None]
    viab = (
        (s_idx[None, None, :] >= smin)
        & (s_idx[None, None, :] <= smax)
        & (s_idx[None, None, :] < L)
    )
    viab = viab.astype(np.float32).reshape(B_LOC, T * SP)

    # gather indices, wrapped per 16-partition group: idx k -> [k%16, k//16]
    exti = np.zeros((128, NTILES * 4), dtype=np.int16)
    ext16 = ext64.astype(np.int16)
    for bq in range(NBQ):
        for g in range(8):
            b = 4 * bq + g // 2
            blk = ext16[b].reshape(4, 16).T  # [pp, w]
            for j in range(NJ):
                k = j * NBQ + bq
                exti[g * 16 : g * 16 + 16, 4 * k : 4 * k + 4] = blk

    return {
        "pred": np.ascontiguousarray(pred, dtype=np.float32),
        "exti": exti,
        "mskip": mskip,
        "minit": minit,
        "mfin": mfin,
        "viab": viab,
    }


def _emit(tc, pred3, exti_ap, mskip_ap, minit_ap, mfin_ap, viab_ap, loss_ap,
          repeats=1):
    nc = tc.nc
    with (
        tc.tile_pool(name="src", bufs=3) as pool_src,
        tc.tile_pool(name="viabp", bufs=2) as pool_vb,
        tc.tile_pool(name="state", bufs=1) as pool_st,
    ):
        sb_exti = pool_st.tile([128, NTILES * 4], I16, name="exti")
        nc.sync.dma_start(sb_exti[:, :], exti_ap[:, :])
        sb_mskip = pool_st.tile([B_LOC, SP], F32, name="mskip")
        nc.sync.dma_start(sb_mskip[:, :], mskip_ap[:, :])
        sb_minit = pool_st.tile([B_LOC, SP], F32, name="minit")
        nc.sync.dma_start(sb_minit[:, :], minit_ap[:, :])
        sb_mfin = pool_st.tile([B_LOC, SP], F32, name="mfin")
        nc.sync.dma_start(sb_mfin[:, :], mfin_ap[:, :])

        sb_bias = pool_st.tile([128, 1], F32, name="biasc")
        nc.vector.memset(sb_bias[:, :], BIAS)

        alpha = pool_st.tile([B_LOC, SP + 2], F32, name="alpha")
        acc = pool_st.tile([B_LOC, 1], F32, name="acc")
        tmp1 = pool_st.tile([B_LOC, SP], F32, name="tmp1")
        tmp2 = pool_st.tile([B_LOC, SP], F32, name="tmp2")
        red = pool_st.tile([B_LOC, 1], F32, name="red")
        rec = pool_st.tile([B_LOC, 1], F32, name="rec")

        stage = [
            pool_st.tile([128, NBQ * SP], F32, name=f"stage{j}") for j in range(NJ)
        ]
        psT = [pool_st.tile([64, 32 * 32], F32, name=f"psT{j}") for j in range(NJ)]
        pbig = [
            pool_st.tile([B_LOC, 32 * SP], F32, name=f"pbig{j}") for j in range(NJ)
        ]

        nc.gpsimd.load_library(library_config.ap_gather)

        for _rep in range(repeats):
            _pipeline(tc, pred3, viab_ap, loss_ap, sb_exti, sb_mskip, sb_minit,
                      sb_mfin, sb_bias, alpha, acc, tmp1, tmp2, red, rec,
                      stage, psT, pbig, pool_src, pool_vb, pool_st)


def _pipeline(tc, pred3, viab_ap, loss_ap, sb_exti, sb_mskip, sb_minit,
              sb_mfin, sb_bias, alpha, acc, tmp1, tmp2, red, rec,
              stage, psT, pbig, pool_src, pool_vb, pool_st):
        nc = tc.nc
        nc.vector.memset(alpha[:, :], 0.0)
        nc.vector.memset(acc[:, :], 0.0)

        for j in range(NJ):
            for bq in range(NBQ):
                k = j * NBQ + bq
                t_src = pool_src.tile([128, C], F32, name="t_src", tag="src")
                nc.sync.dma_start(
                    t_src[:, :],
                    pred3[4 * bq : 4 * bq + 4, 32 * j : 32 * (j + 1), :],
                )
                t_g = pool_src.tile([128, SP], F32, name="t_g", tag="gath")
                nc.gpsimd.ap_gather(
                    out_ap=t_g[:, :],
                    in_ap=t_src[:, :],
                    idxs_ap=sb_exti[:, 4 * k : 4 * k + 4],
                    channels=128,
                    num_elems=C,
                    d=1,
                    num_idxs=SP,
                )
                nc.scalar.activation(
                    stage[j][:, SP * bq : SP * (bq + 1)],
                    t_g[:, :],
                    mybir.ActivationFunctionType.Exp,
                    bias=sb_bias[:, :],
                    scale=1.0,
                )

            psTv = psT[j][:, :].rearrange("p (u b) -> p u b", b=32)
            stgv = stage[j][:, :].rearrange("p (bq s) -> p bq s", s=SP)
            for b in range(B_LOC):
                r, bq = b % 4, b // 4
                for h in range(2):
                    nc.vector.transpose(
                        psTv[32 * h : 32 * (h + 1), :, b],
                        stgv[32 * r : 32 * (r + 1), bq, 32 * h : 32 * (h + 1)],
                    )

            for u in range(32):
                for h in range(2):
                    nc.vector.transpose(
                        pbig[j][:, SP * u + 32 * h : SP * u + 32 * (h + 1)],
                        psTv[32 * h : 32 * (h + 1), u, :],
                    )

            t_vb = pool_vb.tile([B_LOC, 32 * SP], F32, name="t_vb", tag="viab")
            nc.sync.dma_start(
                t_vb[:, :], viab_ap[:, 32 * SP * j : 32 * SP * (j + 1)]
            )
            nc.vector.tensor_tensor(
                pbig[j][:, :], pbig[j][:, :], t_vb[:, :], op=mybir.AluOpType.mult
            )

            for u in range(32):
                t = 32 * j + u
                p_t = pbig[j][:, SP * u : SP * (u + 1)]
                a_cur = alpha[:, 2 : SP + 2]
                if t == 0:
                    nc.vector.tensor_tensor(
                        a_cur, p_t, sb_minit[:, :], op=mybir.AluOpType.mult
                    )
                else:
                    nc.vector.tensor_tensor(
                        tmp1[:, :], alpha[:, 1 : SP + 1], a_cur,
                        op=mybir.AluOpType.add,
                    )
                    nc.vector.tensor_tensor(
                        tmp2[:, :], alpha[:, 0:SP], sb_mskip[:, :],
                        op=mybir.AluOpType.mult,
                    )
                    nc.vector.tensor_tensor(
                        tmp1[:, :], tmp1[:, :], tmp2[:, :], op=mybir.AluOpType.add
                    )
                    nc.vector.tensor_tensor(
                        a_cur, tmp1[:, :], p_t, op=mybir.AluOpType.mult
                    )
                if t % RENORM == RENORM - 1:
                    nc.vector.tensor_reduce(
                        red[:, :], a_cur, axis=mybir.AxisListType.X,
                        op=mybir.AluOpType.max,
                    )
                    nc.vector.reciprocal(rec[:, :], red[:, :])
                    nc.vector.tensor_scalar_mul(a_cur, a_cur, rec[:, :])
                    nc.scalar.activation(
                        red[:, :], red[:, :], mybir.ActivationFunctionType.Ln
                    )
                    nc.vector.tensor_tensor(
                        acc[:, :], acc[:, :], red[:, :], op=mybir.AluOpType.add
                    )

        nc.vector.scalar_tensor_tensor(
            tmp2[:, :], alpha[:, 2 : SP + 2], 1.0, sb_mfin[:, :],
            op0=mybir.AluOpType.bypass, op1=mybir.AluOpType.mult,
            accum_out=red[:, :],
        )
        loss_sb = pool_st.tile([B_LOC, 1], F32, name="loss_sb")
        nc.scalar.activation(
            loss_sb[:, :], red[:, :], mybir.ActivationFunctionType.Ln
        )
        nc.vector.tensor_tensor(
            loss_sb[:, :], loss_sb[:, :], acc[:, :], op=mybir.AluOpType.add
        )
        nc.vector.tensor_scalar(
            loss_sb[:, :], loss_sb[:, :], -1.0, BIAS * T,
            op0=mybir.AluOpType.mult, op1=mybir.AluOpType.add,
        )
        nc.sync.dma_start(loss_ap[:, :], loss_sb[:, :])


_CACHED_NC = None


def build_nc(repeats=1):
    global _CACHED_NC
    if _CACHED_NC is not None and repeats == 1:
        return _CACHED_NC
    nc = bacc.Bacc("TRN2", target_bir_lowering=False, debug=False,
                   num_devices=N_CORES)
    pred = nc.dram_tensor("pred", [B_LOC, T, C], F32, kind="ExternalInput").ap()
    exti = nc.dram_tensor("exti", [128, NTILES * 4], I16,
                          kind="ExternalInput").ap()
    mskip = nc.dram_tensor("mskip", [B_LOC, SP], F32, kind="ExternalInput").ap()
    minit = nc.dram_tensor("minit", [B_LOC, SP], F32, kind="ExternalInput").ap()
    mfin = nc.dram_tensor("mfin", [B_LOC, SP], F32, kind="ExternalInput").ap()
    viab = nc.dram_tensor("viab", [B_LOC, T * SP], F32, kind="ExternalInput").ap()
    loss = nc.dram_tensor("loss", [B_LOC, 1], F32, kind="ExternalOutput").ap()
    with tile.TileContext(nc) as tc:
        _emit(tc, pred, exti, mskip, minit, mfin, viab, loss, repeats=repeats)
    nc.compile()
    if repeats == 1:
        _CACHED_NC = nc
    return nc


def make_in_maps(predicts, labels, label_lengths):
    in_maps = []
    for c in range(N_CORES):
        sl = slice(c * B_LOC, (c + 1) * B_LOC)
        in_maps.append(
            _prep_core_inputs(predicts[sl], labels[sl], label_lengths[sl])
        )
    return in_maps


def kernel(predicts, labels, label_lengths):
    predicts = np.asarray(predicts, dtype=np.float32)
    labels = np.asarray(labels)
    label_lengths = np.asarray(label_lengths)
    nc = build_nc()
    in_maps = make_in_maps(predicts, labels, label_lengths)
    res = run_bass_kernel_spmd(nc, in_maps, core_ids=list(range(N_CORES)))
    losses = np.concatenate(
        [res.results[c]["loss"].reshape(B_LOC) for c in range(N_CORES)]
    )
    return np.float32(losses.mean())
